# revision 44
# baseline (speedup 1.0000x reference)
"""Trainium2 Bass kernel for nn_AdaptiveAttentionLoss (weighted-CE segment mean).

reference semantics (C=2, G=4096, BETA=2):
    ce  = logsumexp(x) - x[label]
    p   = exp(-ce)
    s   = (1 - p^2) * ce          # per-sample weighted CE
    out = mean_over_present_groups( segment_mean(s, index) )

Strategy: data-parallel over samples on 8 NeuronCores. Each core:
  - streams its shard, computes s elementwise (ACT exp/log, DVE arithmetic)
  - builds 64-wide hi/lo one-hots per 128-sample column (tensor_scalar is_equal)
  - one PE matmul per column accumulates [counts | s-sums] for all 4096 groups
    into a PSUM tile ([hi | hi*s]^T @ lo -> [128, 64])
  - AllReduce of the [128, 64] stats, then the masked group-mean average.
"""

import math
from contextlib import ExitStack

import numpy as np

import concourse.bass as bass
import concourse.tile as tile
from concourse import bacc, mybir
from concourse.bass_utils import run_bass_kernel_spmd

F32 = mybir.dt.float32
BF16 = mybir.dt.bfloat16
I32 = mybir.dt.int32
I16 = mybir.dt.int16

N_FULL = 16777216
G = 4096
CORES = 8
P = 128
H = 64  # hi bins (index >> 6)
L = 64  # lo bins (index & 63)

AX = mybir.AxisListType
OP = mybir.AluOpType
ACTF = mybir.ActivationFunctionType


def build_nc(n_core: int, chunk_f: int):
    """Build the SPMD Bass graph for one core holding n_core samples."""
    assert n_core % (P * chunk_f) == 0
    ftot = n_core // P
    nchunk = ftot // chunk_f

    nc = bacc.Bacc("TRN2", target_bir_lowering=False, debug=False)

    x_d = nc.declare_dram_parameter("x", [n_core, 2], F32, isOutput=False)
    # int64 inputs are passed from the host as pairs of int32 words (same bytes)
    idx_d = nc.declare_dram_parameter("index", [n_core, 2], I32, isOutput=False)
    lab_d = nc.declare_dram_parameter("label", [n_core, 2], I32, isOutput=False)
    out_d = nc.declare_dram_parameter("out", [1, 1], F32, isOutput=True)

    cc_in = nc.dram_tensor("cc_in", [P, L], F32)
    cc_out = nc.dram_tensor("cc_out", [P, L], F32, addr_space="Shared")

    x_v = x_d.ap().rearrange("(p f) c -> p f c", p=P)      # [128, ftot, 2]
    idx_v = idx_d.ap().rearrange("(p f) c -> p f c", p=P)
    lab_v = lab_d.ap().rearrange("(p f) c -> p f c", p=P)

    with tile.TileContext(nc) as tc, ExitStack() as ctx:
        const_pool = ctx.enter_context(tc.tile_pool(name="const", bufs=1))
        in_pool = ctx.enter_context(tc.tile_pool(name="inp", bufs=2))
        scr_pool = ctx.enter_context(tc.tile_pool(name="scr", bufs=1))
        ew_pool = ctx.enter_context(tc.tile_pool(name="ew", bufs=2))
        oh_pool = ctx.enter_context(tc.tile_pool(name="oh", bufs=3))
        fin_pool = ctx.enter_context(tc.tile_pool(name="fin", bufs=1))
        psum_pool = ctx.enter_context(
            tc.tile_pool(name="psum", bufs=1, space="PSUM")
        )

        # iota row 0..63, identical in every partition (int16 -> bf16 copy;
        # values < 64 are exact in bf16, and bf16 in/out lets ts hit 4x mode)
        iota_i = const_pool.tile([P, H], I16)
        iota_t = const_pool.tile([P, H], BF16)
        nc.gpsimd.iota(iota_i[:], pattern=[[1, H]], base=0, channel_multiplier=0)
        nc.vector.tensor_copy(out=iota_t[:], in_=iota_i[:])
        RB = 64  # tiles per DVE batch (inner dim of the bin-major layout)
        iotw_i = const_pool.tile([P, 2 * H * RB], I16)
        iotw_t = const_pool.tile([P, 2 * H * RB], BF16)
        nc.gpsimd.iota(iotw_i[:], pattern=[[0, 2], [1, H], [0, RB]], base=0,
                       channel_multiplier=0)
        nc.vector.tensor_copy(out=iotw_t[:], in_=iotw_i[:])

        hist = psum_pool.tile([P, L], F32)  # [(cnt h | val h), lo]

        n_tiles_total = ftot  # one matmul per free column
        tile_no = 0

        for c in range(nchunk):
            sl = slice(c * chunk_f, (c + 1) * chunk_f)
            xt = in_pool.tile([P, chunk_f, 2], F32, tag="x")
            it = in_pool.tile([P, chunk_f, 2], I32, tag="idx")
            lt = in_pool.tile([P, chunk_f, 2], I32, tag="lab")
            nc.sync.dma_start(out=xt[:], in_=x_v[:, sl, :])
            nc.sync.dma_start(out=it[:], in_=idx_v[:, sl, :])
            nc.sync.dma_start(out=lt[:], in_=lab_v[:, sl, :])

            x0 = xt[:, :, 0]
            x1 = xt[:, :, 1]
            idx_w = it[:, :, 0]  # low int32 word of the int64
            lab_w = lt[:, :, 0]

            d = scr_pool.tile([P, chunk_f], F32, tag="d")
            sign = scr_pool.tile([P, chunk_f], F32, tag="sign")
            t = scr_pool.tile([P, chunk_f], F32, tag="t")
            e = scr_pool.tile([P, chunk_f], F32, tag="e")
            ce = scr_pool.tile([P, chunk_f], F32, tag="ce")
            p = scr_pool.tile([P, chunk_f], F32, tag="p")
            w = scr_pool.tile([P, chunk_f], F32, tag="w")
            sv = ew_pool.tile([P, chunk_f], BF16, tag="sv")
            hi_tt = ew_pool.tile([P, chunk_f], BF16, tag="hi")
            lo_tt = ew_pool.tile([P, chunk_f], BF16, tag="lo")
            hi_t = hi_tt[:]
            lo_t = lo_tt[:]

            nc.vector.tensor_tensor(out=d[:], in0=x0, in1=x1, op=OP.subtract)
            # sign = 1 - 2*label   (int32 read converted by the fp32 ALU)
            nc.vector.tensor_scalar(
                out=sign[:], in0=lab_w, scalar1=-2.0, scalar2=1.0,
                op0=OP.mult, op1=OP.add,
            )
            nc.vector.tensor_tensor(out=t[:], in0=d[:], in1=sign[:], op=OP.mult)
            # e = exp(-t); ce = ln(1+e); p = exp(-ce) = softmax prob of label
            # (Exp and Ln share the natural_log_exp_and_others ACT table set)
            nc.scalar.activation(e[:], t[:], ACTF.Exp, scale=-1.0)
            nc.scalar.activation(ce[:], e[:], ACTF.Ln, bias=1.0)
            nc.scalar.activation(p[:], ce[:], ACTF.Exp, scale=-1.0)
            # s = (1 - p^2) * ce
            nc.vector.tensor_tensor(out=w[:], in0=p[:], in1=p[:], op=OP.mult)
            nc.vector.tensor_scalar(
                out=w[:], in0=w[:], scalar1=-1.0, scalar2=1.0,
                op0=OP.mult, op1=OP.add,
            )
            nc.vector.tensor_tensor(out=sv[:], in0=w[:], in1=ce[:], op=OP.mult)
            # hi = index >> 6 (int shift, then cast), lo = index - 64*hi
            hi_i = scr_pool.tile([P, chunk_f], I32, tag="hi_i")
            idxf = scr_pool.tile([P, chunk_f], F32, tag="idxf")
            nc.vector.tensor_scalar(
                out=hi_i[:], in0=idx_w, scalar1=6, scalar2=None,
                op0=OP.logical_shift_right,
            )
            nc.vector.tensor_copy(out=hi_t, in_=hi_i[:])
            nc.vector.tensor_copy(out=idxf[:], in_=idx_w)
            nc.vector.scalar_tensor_tensor(
                out=lo_t, in0=hi_t, scalar=-64.0, in1=idxf[:],
                op0=OP.mult, op1=OP.add,
            )

            # Histogram: bin-major one-hot batches. ohb holds the
            # [hi | hi*s] pair (matmul lhsT), olb the lo one-hot (rhs).
            # All TT operands keep innermost step-1 (2x-eligible, bf16).
            R = min(RB, chunk_f)
            for b in range(chunk_f // R):
                bsl = slice(b * R, (b + 1) * R)
                ohb = oh_pool.tile([P, 2, H, R], BF16, tag="ohb")
                olb = oh_pool.tile([P, H, R], BF16, tag="olb")
                iota_rep = iotw_t[:, 0 : H * R].rearrange("p (h r) -> p h r", r=R)
                hi_rep = hi_t[:, bsl].unsqueeze(1).broadcast_to((P, H, R))
                lo_rep = lo_t[:, bsl].unsqueeze(1).broadcast_to((P, H, R))
                sv_rep = sv[:, bsl].unsqueeze(1).broadcast_to((P, H, R))
                nc.vector.tensor_tensor(
                    out=ohb[:, 0, :, :], in0=iota_rep, in1=hi_rep, op=OP.is_equal
                )
                import os as _os2
                _mv = int(_os2.environ.get("KMULT", "0"))
                if _mv == 1:
                    nc.vector.tensor_tensor(
                        out=ohb[:, 1, :, :], in0=sv_rep, in1=ohb[:, 0, :, :],
                        op=OP.mult,
                    )
                elif _mv == 2:
                    nc.vector.scalar_tensor_tensor(
                        out=ohb[:, 1, :, :], in0=ohb[:, 0, :, :], scalar=1.0,
                        in1=sv_rep, op0=OP.mult, op1=OP.mult,
                    )
                else:
                    nc.vector.tensor_tensor(
                        out=ohb[:, 1, :, :], in0=ohb[:, 0, :, :], in1=sv_rep,
                        op=OP.mult,
                    )
                nc.vector.tensor_tensor(
                    out=olb[:], in0=iota_rep, in1=lo_rep, op=OP.is_equal
                )
                import os as _os
                _kp = int(_os.environ.get("KPROBE", "0"))
                _js = range(R) if not _kp else range(1)
                for j in _js:
                    nc.tensor.matmul(
                        out=hist[:], lhsT=ohb[:, :, :, j], rhs=olb[:, :, j],
                        start=(tile_no == 0),
                        stop=(tile_no + (R if _kp else 1) >= n_tiles_total),
                    )
                    tile_no += R if _kp else 1

        # ---- finalize: AllReduce the [128, 64] stats, then masked mean ----
        stats = fin_pool.tile([P, L], F32, tag="stats")
        nc.vector.tensor_copy(out=stats[:], in_=hist[:])
        nc.sync.dma_start(out=cc_in.ap(), in_=stats[:])
        nc.gpsimd.collective_compute(
            "AllReduce",
            OP.add,
            ins=[cc_in.ap().opt()],
            outs=[cc_out.ap().opt()],
            replica_groups=[list(range(CORES))],
        )
        cnt_t = fin_pool.tile([H, L], F32, tag="cnt_t")
        val_t = fin_pool.tile([H, L], F32, tag="val_t")
        cc_v = cc_out.ap()
        nc.sync.dma_start(out=cnt_t[:], in_=cc_v[0:H, :])
        nc.sync.dma_start(out=val_t[:], in_=cc_v[H : 2 * H, :])
        cnt = cnt_t[:]
        val = val_t[:]
        cntc = fin_pool.tile([H, L], F32, tag="cntc")
        gm = fin_pool.tile([H, L], F32, tag="gm")
        pres = fin_pool.tile([H, L], F32, tag="pres")
        nc.vector.tensor_scalar(
            out=cntc[:], in0=cnt, scalar1=1.0, scalar2=None, op0=OP.max
        )
        nc.vector.reciprocal(out=cntc[:], in_=cntc[:])
        nc.vector.tensor_tensor(out=gm[:], in0=val, in1=cntc[:], op=OP.mult)
        nc.vector.tensor_scalar(
            out=pres[:], in0=cnt, scalar1=0.0, scalar2=None, op0=OP.is_gt
        )
        nc.vector.tensor_tensor(out=gm[:], in0=gm[:], in1=pres[:], op=OP.mult)

        tot = fin_pool.tile([1, 1], F32, tag="tot")
        npres = fin_pool.tile([1, 1], F32, tag="npres")
        ans = fin_pool.tile([1, 1], F32, tag="ans")
        nc.gpsimd.tensor_reduce(out=tot[:], in_=gm[:], axis=AX.XYZWC, op=OP.add)
        nc.gpsimd.tensor_reduce(out=npres[:], in_=pres[:], axis=AX.XYZWC, op=OP.add)
        nc.vector.reciprocal(out=npres[:], in_=npres[:])
        nc.vector.tensor_tensor(out=ans[:], in0=tot[:], in1=npres[:], op=OP.mult)
        nc.sync.dma_start(out=out_d.ap(), in_=ans[:])

    nc.finalize()
    return nc


def make_in_maps(x, index, label, n_cores=CORES):
    n = x.shape[0]
    nc_sz = n // n_cores
    xs = np.ascontiguousarray(x, dtype=np.float32)
    iv = np.ascontiguousarray(index, dtype=np.int64).view(np.int32).reshape(n, 2)
    lv = np.ascontiguousarray(label, dtype=np.int64).view(np.int32).reshape(n, 2)
    maps = []
    for k in range(n_cores):
        sl = slice(k * nc_sz, (k + 1) * nc_sz)
        maps.append(
            {
                "x": np.ascontiguousarray(xs[sl]),
                "index": np.ascontiguousarray(iv[sl]),
                "label": np.ascontiguousarray(lv[sl]),
            }
        )
    return maps


_NC_CACHE = {}


def _get_nc(n_core, chunk_f):
    key = (n_core, chunk_f)
    if key not in _NC_CACHE:
        _NC_CACHE[key] = build_nc(n_core, chunk_f)
    return _NC_CACHE[key]


def kernel(x, index, label):
    n = x.shape[0]
    n_core = n // CORES
    nc = _get_nc(n_core, min(1024, n_core // P))
    in_maps = make_in_maps(x, index, label)
    res = run_bass_kernel_spmd(nc, in_maps, core_ids=list(range(CORES)))
    return np.float32(res.results[0]["out"][0, 0])


if __name__ == "__main__":
    rng = np.random.default_rng(0)
    n = 128 * 32 * CORES
    x = rng.standard_normal((n, 2), dtype=np.float32)
    index = rng.integers(0, G, n, dtype=np.int64)
    label = rng.integers(0, 2, n, dtype=np.int64)
    got = kernel(x, index, label)
    # numpy reference
    m = np.maximum(x[:, 0], x[:, 1])
    logz = m + np.log(np.exp(x[:, 0] - m) + np.exp(x[:, 1] - m))
    xt = x[np.arange(n), label]
    ce = logz - xt
    p = np.exp(xt - logz)
    s = (1.0 - p**2) * ce
    seg = np.zeros(G)
    cntr = np.zeros(G)
    np.add.at(seg, index, s)
    np.add.at(cntr, index, 1.0)
    pres = cntr > 0
    gmean = np.where(pres, seg / np.maximum(cntr, 1), 0.0)
    want = gmean.sum() / pres.sum()
    print("got", got, "want", want, "rel", abs(got - want) / abs(want))


# revision 45
# speedup vs baseline: 1.1048x; 1.1048x over previous
"""Trainium2 Bass kernel for nn_AdaptiveAttentionLoss (weighted-CE segment mean).

reference semantics (C=2, G=4096, BETA=2):
    ce  = logsumexp(x) - x[label]
    p   = exp(-ce)
    s   = (1 - p^2) * ce          # per-sample weighted CE
    out = mean_over_present_groups( segment_mean(s, index) )

Strategy: data-parallel over samples on 8 NeuronCores. Each core:
  - streams its shard, computes s elementwise (ACT exp/log, DVE arithmetic)
  - builds 64-wide hi/lo one-hots per 128-sample column (tensor_scalar is_equal)
  - one PE matmul per column accumulates [counts | s-sums] for all 4096 groups
    into a PSUM tile ([hi | hi*s]^T @ lo -> [128, 64])
  - AllReduce of the [128, 64] stats, then the masked group-mean average.
"""

from contextlib import ExitStack

import numpy as np

import concourse.bass as bass
import concourse.tile as tile
from concourse import bacc, mybir
from concourse.bass_utils import run_bass_kernel_spmd

F32 = mybir.dt.float32
BF16 = mybir.dt.bfloat16
I32 = mybir.dt.int32
I16 = mybir.dt.int16

N_FULL = 16777216
G = 4096
CORES = 8
P = 128
H = 64  # hi bins (index >> 6)
L = 64  # lo bins (index & 63)

AX = mybir.AxisListType
OP = mybir.AluOpType
ACTF = mybir.ActivationFunctionType


def build_nc(n_core: int, chunk_f: int):
    """Build the SPMD Bass graph for one core holding n_core samples."""
    assert n_core % (P * chunk_f) == 0
    ftot = n_core // P
    nchunk = ftot // chunk_f

    nc = bacc.Bacc("TRN2", target_bir_lowering=False, debug=False)

    x_d = nc.declare_dram_parameter("x", [n_core, 2], F32, isOutput=False)
    # int64 inputs are passed from the host as pairs of int32 words (same bytes)
    idx_d = nc.declare_dram_parameter("index", [n_core, 2], I32, isOutput=False)
    lab_d = nc.declare_dram_parameter("label", [n_core, 2], I32, isOutput=False)
    out_d = nc.declare_dram_parameter("out", [1, 1], F32, isOutput=True)

    cc_in = nc.dram_tensor("cc_in", [P, L], F32)
    cc_out = nc.dram_tensor("cc_out", [P, L], F32, addr_space="Shared")

    x_v = x_d.ap().rearrange("(p f) c -> p f c", p=P)      # [128, ftot, 2]
    idx_v = idx_d.ap().rearrange("(p f) c -> p f c", p=P)
    lab_v = lab_d.ap().rearrange("(p f) c -> p f c", p=P)

    with tile.TileContext(nc) as tc, ExitStack() as ctx:
        const_pool = ctx.enter_context(tc.tile_pool(name="const", bufs=1))
        in_pool = ctx.enter_context(tc.tile_pool(name="inp", bufs=2))
        scr_pool = ctx.enter_context(tc.tile_pool(name="scr", bufs=1))
        ew_pool = ctx.enter_context(tc.tile_pool(name="ew", bufs=2))
        oh_pool = ctx.enter_context(tc.tile_pool(name="oh", bufs=4))
        fin_pool = ctx.enter_context(tc.tile_pool(name="fin", bufs=1))
        psum_pool = ctx.enter_context(
            tc.tile_pool(name="psum", bufs=1, space="PSUM")
        )

        # iota row 0..63, identical in every partition (int16 -> bf16 copy;
        # values < 64 are exact in bf16, and bf16 in/out lets ts hit 4x mode)
        iota_i = const_pool.tile([P, H], I16)
        iota_t = const_pool.tile([P, H], BF16)
        nc.gpsimd.iota(iota_i[:], pattern=[[1, H]], base=0, channel_multiplier=0)
        nc.vector.tensor_copy(out=iota_t[:], in_=iota_i[:])
        RB = 32  # tiles per DVE batch (inner dim of the bin-major layout)
        iotw_i = const_pool.tile([P, 2 * H * RB], I16)
        iotw_t = const_pool.tile([P, 2 * H * RB], BF16)
        nc.gpsimd.iota(iotw_i[:], pattern=[[0, 2], [1, H], [0, RB]], base=0,
                       channel_multiplier=0)
        nc.vector.tensor_copy(out=iotw_t[:], in_=iotw_i[:])

        hist = psum_pool.tile([P, L], F32)  # [(cnt h | val h), lo]

        n_tiles_total = ftot  # one matmul per free column
        tile_no = 0

        for c in range(nchunk):
            sl = slice(c * chunk_f, (c + 1) * chunk_f)
            xt = in_pool.tile([P, chunk_f, 2], F32, tag="x")
            it = in_pool.tile([P, chunk_f, 2], I32, tag="idx")
            lt = in_pool.tile([P, chunk_f, 2], I32, tag="lab")
            nc.sync.dma_start(out=xt[:], in_=x_v[:, sl, :])
            nc.sync.dma_start(out=it[:], in_=idx_v[:, sl, :])
            nc.sync.dma_start(out=lt[:], in_=lab_v[:, sl, :])

            x0 = xt[:, :, 0]
            x1 = xt[:, :, 1]
            idx_w = it[:, :, 0]  # low int32 word of the int64
            lab_w = lt[:, :, 0]

            d = scr_pool.tile([P, chunk_f], F32, tag="d")
            sign = scr_pool.tile([P, chunk_f], F32, tag="sign")
            t = scr_pool.tile([P, chunk_f], F32, tag="t")
            e = scr_pool.tile([P, chunk_f], F32, tag="e")
            ce = scr_pool.tile([P, chunk_f], F32, tag="ce")
            p = scr_pool.tile([P, chunk_f], F32, tag="p")
            w = scr_pool.tile([P, chunk_f], F32, tag="w")
            sv = ew_pool.tile([P, chunk_f], BF16, tag="sv")
            hi_tt = ew_pool.tile([P, chunk_f], BF16, tag="hi")
            lo_tt = ew_pool.tile([P, chunk_f], BF16, tag="lo")
            hi_t = hi_tt[:]
            lo_t = lo_tt[:]

            nc.vector.tensor_tensor(out=d[:], in0=x0, in1=x1, op=OP.subtract)
            # sign = 1 - 2*label   (int32 read converted by the fp32 ALU)
            nc.vector.tensor_scalar(
                out=sign[:], in0=lab_w, scalar1=-2.0, scalar2=1.0,
                op0=OP.mult, op1=OP.add,
            )
            nc.vector.tensor_tensor(out=t[:], in0=d[:], in1=sign[:], op=OP.mult)
            # e = exp(-t); ce = ln(1+e); p = exp(-ce) = softmax prob of label
            # (Exp and Ln share the natural_log_exp_and_others ACT table set)
            nc.scalar.activation(e[:], t[:], ACTF.Exp, scale=-1.0)
            nc.scalar.activation(ce[:], e[:], ACTF.Ln, bias=1.0)
            nc.scalar.activation(p[:], ce[:], ACTF.Exp, scale=-1.0)
            # s = (1 - p^2) * ce
            nc.vector.tensor_tensor(out=w[:], in0=p[:], in1=p[:], op=OP.mult)
            nc.vector.tensor_scalar(
                out=w[:], in0=w[:], scalar1=-1.0, scalar2=1.0,
                op0=OP.mult, op1=OP.add,
            )
            nc.vector.tensor_tensor(out=sv[:], in0=w[:], in1=ce[:], op=OP.mult)
            # hi = index >> 6 (int shift, then cast), lo = index - 64*hi
            hi_i = scr_pool.tile([P, chunk_f], I32, tag="hi_i")
            idxf = scr_pool.tile([P, chunk_f], F32, tag="idxf")
            nc.vector.tensor_scalar(
                out=hi_i[:], in0=idx_w, scalar1=6, scalar2=None,
                op0=OP.logical_shift_right,
            )
            nc.vector.tensor_copy(out=hi_t, in_=hi_i[:])
            nc.vector.tensor_copy(out=idxf[:], in_=idx_w)
            nc.vector.scalar_tensor_tensor(
                out=lo_t, in0=hi_t, scalar=-64.0, in1=idxf[:],
                op0=OP.mult, op1=OP.add,
            )

            # Histogram: bin-major one-hot batches. ohb holds the
            # [hi | hi*s] pair (matmul lhsT), olb the lo one-hot (rhs).
            # All TT operands keep innermost step-1 (2x-eligible, bf16).
            R = min(RB, chunk_f)
            for b in range(chunk_f // R):
                bsl = slice(b * R, (b + 1) * R)
                ohb = oh_pool.tile([P, 2, H, R], BF16, tag="ohb")
                olb = oh_pool.tile([P, H, R], BF16, tag="olb")
                iota_rep = iotw_t[:, 0 : H * R].rearrange("p (h r) -> p h r", r=R)
                hi_rep = hi_t[:, bsl].unsqueeze(1).broadcast_to((P, H, R))
                lo_rep = lo_t[:, bsl].unsqueeze(1).broadcast_to((P, H, R))
                sv_rep = sv[:, bsl].unsqueeze(1).broadcast_to((P, H, R))
                nc.vector.tensor_tensor(
                    out=ohb[:, 0, :, :], in0=iota_rep, in1=hi_rep, op=OP.is_equal
                )
                nc.vector.tensor_tensor(
                    out=ohb[:, 1, :, :], in0=ohb[:, 0, :, :], in1=sv_rep,
                    op=OP.mult,
                )
                nc.vector.tensor_tensor(
                    out=olb[:], in0=iota_rep, in1=lo_rep, op=OP.is_equal
                )
                for j in range(R):
                    nc.tensor.matmul(
                        out=hist[:], lhsT=ohb[:, :, :, j], rhs=olb[:, :, j],
                        start=(tile_no == 0),
                        stop=(tile_no == n_tiles_total - 1),
                    )
                    tile_no += 1

        # ---- finalize: AllReduce the [128, 64] stats, then masked mean ----
        stats = fin_pool.tile([P, L], F32, tag="stats")
        nc.vector.tensor_copy(out=stats[:], in_=hist[:])
        nc.sync.dma_start(out=cc_in.ap(), in_=stats[:])
        nc.gpsimd.collective_compute(
            "AllReduce",
            OP.add,
            ins=[cc_in.ap().opt()],
            outs=[cc_out.ap().opt()],
            replica_groups=[list(range(CORES))],
        )
        cnt_t = fin_pool.tile([H, L], F32, tag="cnt_t")
        val_t = fin_pool.tile([H, L], F32, tag="val_t")
        cc_v = cc_out.ap()
        nc.sync.dma_start(out=cnt_t[:], in_=cc_v[0:H, :])
        nc.sync.dma_start(out=val_t[:], in_=cc_v[H : 2 * H, :])
        cnt = cnt_t[:]
        val = val_t[:]
        cntc = fin_pool.tile([H, L], F32, tag="cntc")
        gm = fin_pool.tile([H, L], F32, tag="gm")
        pres = fin_pool.tile([H, L], F32, tag="pres")
        nc.vector.tensor_scalar(
            out=cntc[:], in0=cnt, scalar1=1.0, scalar2=None, op0=OP.max
        )
        nc.vector.reciprocal(out=cntc[:], in_=cntc[:])
        nc.vector.tensor_tensor(out=gm[:], in0=val, in1=cntc[:], op=OP.mult)
        nc.vector.tensor_scalar(
            out=pres[:], in0=cnt, scalar1=0.0, scalar2=None, op0=OP.is_gt
        )
        nc.vector.tensor_tensor(out=gm[:], in0=gm[:], in1=pres[:], op=OP.mult)

        tot = fin_pool.tile([1, 1], F32, tag="tot")
        npres = fin_pool.tile([1, 1], F32, tag="npres")
        ans = fin_pool.tile([1, 1], F32, tag="ans")
        nc.gpsimd.tensor_reduce(out=tot[:], in_=gm[:], axis=AX.XYZWC, op=OP.add)
        nc.gpsimd.tensor_reduce(out=npres[:], in_=pres[:], axis=AX.XYZWC, op=OP.add)
        nc.vector.reciprocal(out=npres[:], in_=npres[:])
        nc.vector.tensor_tensor(out=ans[:], in0=tot[:], in1=npres[:], op=OP.mult)
        nc.sync.dma_start(out=out_d.ap(), in_=ans[:])

    nc.finalize()
    return nc


def make_in_maps(x, index, label, n_cores=CORES):
    n = x.shape[0]
    nc_sz = n // n_cores
    xs = np.ascontiguousarray(x, dtype=np.float32)
    iv = np.ascontiguousarray(index, dtype=np.int64).view(np.int32).reshape(n, 2)
    lv = np.ascontiguousarray(label, dtype=np.int64).view(np.int32).reshape(n, 2)
    maps = []
    for k in range(n_cores):
        sl = slice(k * nc_sz, (k + 1) * nc_sz)
        maps.append(
            {
                "x": np.ascontiguousarray(xs[sl]),
                "index": np.ascontiguousarray(iv[sl]),
                "label": np.ascontiguousarray(lv[sl]),
            }
        )
    return maps


_NC_CACHE = {}


def _get_nc(n_core, chunk_f):
    key = (n_core, chunk_f)
    if key not in _NC_CACHE:
        _NC_CACHE[key] = build_nc(n_core, chunk_f)
    return _NC_CACHE[key]


def kernel(x, index, label):
    n = x.shape[0]
    n_core = n // CORES
    nc = _get_nc(n_core, min(1024, n_core // P))
    in_maps = make_in_maps(x, index, label)
    res = run_bass_kernel_spmd(nc, in_maps, core_ids=list(range(CORES)))
    return np.float32(res.results[0]["out"][0, 0])


if __name__ == "__main__":
    rng = np.random.default_rng(0)
    n = 128 * 32 * CORES
    x = rng.standard_normal((n, 2), dtype=np.float32)
    index = rng.integers(0, G, n, dtype=np.int64)
    label = rng.integers(0, 2, n, dtype=np.int64)
    got = kernel(x, index, label)
    # numpy reference
    m = np.maximum(x[:, 0], x[:, 1])
    logz = m + np.log(np.exp(x[:, 0] - m) + np.exp(x[:, 1] - m))
    xt = x[np.arange(n), label]
    ce = logz - xt
    p = np.exp(xt - logz)
    s = (1.0 - p**2) * ce
    seg = np.zeros(G)
    cntr = np.zeros(G)
    np.add.at(seg, index, s)
    np.add.at(cntr, index, 1.0)
    pres = cntr > 0
    gmean = np.where(pres, seg / np.maximum(cntr, 1), 0.0)
    want = gmean.sum() / pres.sum()
    print("got", got, "want", want, "rel", abs(got - want) / abs(want))


# revision 46
# speedup vs baseline: 1.4119x; 1.2779x over previous
"""Trainium2 Bass kernel for nn_AdaptiveAttentionLoss (weighted-CE segment mean).

reference semantics (C=2, G=4096, BETA=2):
    ce  = logsumexp(x) - x[label]
    p   = exp(-ce)
    s   = (1 - p^2) * ce          # per-sample weighted CE
    out = mean_over_present_groups( segment_mean(s, index) )

Strategy: data-parallel over the sample dim on 8 NeuronCores. Each core:
  - streams its shard (x f32 pairs; index/label int64 passed as int32 word
    pairs), computes s elementwise (ACT exp/ln, DVE arithmetic)
  - segment-reduce via two-level one-hots: index = 64*hi + lo. For batches
    of 32 sample-columns the DVE builds bin-major one-hot tiles (bf16,
    innermost step-1 so the 2x perf mode applies):
        ohb[p, 0, h, r] = (hi[p,r] == h),  ohb[p, 1, h, r] = ohb0 * s
        olb[p, h, r]    = (lo[p,r] == l)
    and one PE matmul per column accumulates [counts | s*sums] for all
    4096 groups into PSUM: hist[{cnt,val}*64h, 64l] += ohb^T @ olb.
  - AllReduce of the [128, 64] stats across cores, then the masked
    group-mean average on-chip; all cores emit the same scalar.
"""

from contextlib import ExitStack

import numpy as np

import concourse.bass as bass
import concourse.tile as tile
from concourse import bacc, mybir
from concourse.bass_utils import run_bass_kernel_spmd

F32 = mybir.dt.float32
BF16 = mybir.dt.bfloat16
I32 = mybir.dt.int32
I16 = mybir.dt.int16

N_FULL = 16777216
G = 4096
CORES = 8
P = 128
H = 64  # hi bins (index >> 6)
L = 64  # lo bins (index & 63)

AX = mybir.AxisListType
OP = mybir.AluOpType
ACTF = mybir.ActivationFunctionType


def build_nc(n_core: int, chunk_f: int):
    """Build the SPMD Bass graph for one core holding n_core samples."""
    assert n_core % (P * chunk_f) == 0
    ftot = n_core // P
    nchunk = ftot // chunk_f

    nc = bacc.Bacc("TRN2", target_bir_lowering=False, debug=False)

    x_d = nc.declare_dram_parameter("x", [n_core, 2], F32, isOutput=False)
    # int64 inputs are passed from the host as pairs of int32 words (same bytes)
    idx_d = nc.declare_dram_parameter("index", [n_core, 2], I32, isOutput=False)
    lab_d = nc.declare_dram_parameter("label", [n_core, 2], I32, isOutput=False)
    out_d = nc.declare_dram_parameter("out", [1, 1], F32, isOutput=True)

    cc_in = nc.dram_tensor("cc_in", [P, L], F32)
    cc_out = nc.dram_tensor("cc_out", [P, L], F32, addr_space="Shared")

    x_v = x_d.ap().rearrange("(p f) c -> p f c", p=P)      # [128, ftot, 2]
    idx_v = idx_d.ap().rearrange("(p f) c -> p f c", p=P)
    lab_v = lab_d.ap().rearrange("(p f) c -> p f c", p=P)

    with tile.TileContext(nc) as tc, ExitStack() as ctx:
        const_pool = ctx.enter_context(tc.tile_pool(name="const", bufs=1))
        in_pool = ctx.enter_context(tc.tile_pool(name="inp", bufs=2))
        scr_pool = ctx.enter_context(tc.tile_pool(name="scr", bufs=1))
        ew_pool = ctx.enter_context(tc.tile_pool(name="ew", bufs=2))
        oh_pool = ctx.enter_context(tc.tile_pool(name="oh", bufs=4))
        fin_pool = ctx.enter_context(tc.tile_pool(name="fin", bufs=1))
        psum_pool = ctx.enter_context(
            tc.tile_pool(name="psum", bufs=1, space="PSUM")
        )

        # iota row 0..63, identical in every partition (int16 -> bf16 copy;
        # values < 64 are exact in bf16, and bf16 in/out lets ts hit 4x mode)
        iota_i = const_pool.tile([P, H], I16)
        iota_t = const_pool.tile([P, H], BF16)
        nc.gpsimd.iota(iota_i[:], pattern=[[1, H]], base=0, channel_multiplier=0)
        nc.vector.tensor_copy(out=iota_t[:], in_=iota_i[:])
        RB = 32  # tiles per DVE batch (inner dim of the bin-major layout)
        iotw_i = const_pool.tile([P, 2 * H * RB], I16)
        iotw_t = const_pool.tile([P, 2 * H * RB], BF16)
        nc.gpsimd.iota(iotw_i[:], pattern=[[0, 2], [1, H], [0, RB]], base=0,
                       channel_multiplier=0)
        nc.vector.tensor_copy(out=iotw_t[:], in_=iotw_i[:])

        hist = psum_pool.tile([P, L], F32)  # [(cnt h | val h), lo]

        n_tiles_total = ftot  # one matmul per free column
        tile_no = 0

        for c in range(nchunk):
            sl = slice(c * chunk_f, (c + 1) * chunk_f)
            xt = in_pool.tile([P, chunk_f, 2], F32, tag="x")
            it = in_pool.tile([P, chunk_f, 2], I32, tag="idx")
            lt = in_pool.tile([P, chunk_f, 2], I32, tag="lab")
            nc.sync.dma_start(out=xt[:], in_=x_v[:, sl, :])
            nc.sync.dma_start(out=it[:], in_=idx_v[:, sl, :])
            nc.sync.dma_start(out=lt[:], in_=lab_v[:, sl, :])

            x0 = xt[:, :, 0]
            x1 = xt[:, :, 1]
            idx_w = it[:, :, 0]  # low int32 word of the int64
            lab_w = lt[:, :, 0]

            d = scr_pool.tile([P, chunk_f], F32, tag="d")
            sign = scr_pool.tile([P, chunk_f], F32, tag="sign")
            t = scr_pool.tile([P, chunk_f], F32, tag="t")
            e = scr_pool.tile([P, chunk_f], F32, tag="e")
            ce = scr_pool.tile([P, chunk_f], F32, tag="ce")
            p = scr_pool.tile([P, chunk_f], F32, tag="p")
            w = scr_pool.tile([P, chunk_f], F32, tag="w")
            sv = ew_pool.tile([P, chunk_f], BF16, tag="sv")
            hi_tt = ew_pool.tile([P, chunk_f], BF16, tag="hi")
            lo_tt = ew_pool.tile([P, chunk_f], BF16, tag="lo")
            hi_t = hi_tt[:]
            lo_t = lo_tt[:]

            nc.vector.tensor_tensor(out=d[:], in0=x0, in1=x1, op=OP.subtract)
            # sign = 1 - 2*label   (int32 read converted by the fp32 ALU)
            nc.vector.tensor_scalar(
                out=sign[:], in0=lab_w, scalar1=-2.0, scalar2=1.0,
                op0=OP.mult, op1=OP.add,
            )
            nc.vector.tensor_tensor(out=t[:], in0=d[:], in1=sign[:], op=OP.mult)
            # e = exp(-t); ce = ln(1+e); p = exp(-ce) = softmax prob of label
            # (Exp and Ln share the natural_log_exp_and_others ACT table set)
            nc.scalar.activation(e[:], t[:], ACTF.Exp, scale=-1.0)
            nc.scalar.activation(ce[:], e[:], ACTF.Ln, bias=1.0)
            nc.scalar.activation(p[:], ce[:], ACTF.Exp, scale=-1.0)
            # s = (1 - p^2) * ce
            nc.vector.tensor_tensor(out=w[:], in0=p[:], in1=p[:], op=OP.mult)
            nc.vector.tensor_scalar(
                out=w[:], in0=w[:], scalar1=-1.0, scalar2=1.0,
                op0=OP.mult, op1=OP.add,
            )
            nc.vector.tensor_tensor(out=sv[:], in0=w[:], in1=ce[:], op=OP.mult)
            # hi = index >> 6 (int shift, then cast), lo = index - 64*hi
            hi_i = scr_pool.tile([P, chunk_f], I32, tag="hi_i")
            idxf = scr_pool.tile([P, chunk_f], F32, tag="idxf")
            nc.vector.tensor_scalar(
                out=hi_i[:], in0=idx_w, scalar1=6, scalar2=None,
                op0=OP.logical_shift_right,
            )
            nc.vector.tensor_copy(out=hi_t, in_=hi_i[:])
            nc.vector.tensor_copy(out=idxf[:], in_=idx_w)
            nc.vector.scalar_tensor_tensor(
                out=lo_t, in0=hi_t, scalar=-64.0, in1=idxf[:],
                op0=OP.mult, op1=OP.add,
            )

            # Histogram: bin-major one-hot batches. ohb holds the
            # [hi | hi*s] pair (matmul lhsT), olb the lo one-hot (rhs).
            # All TT operands keep innermost step-1 (2x-eligible, bf16).
            R = min(RB, chunk_f)
            for b in range(chunk_f // R):
                bsl = slice(b * R, (b + 1) * R)
                ohb = oh_pool.tile([P, 2, H, R], BF16, tag="ohb")
                olb = oh_pool.tile([P, H, R], BF16, tag="olb")
                iota_rep = iotw_t[:, 0 : H * R].rearrange("p (h r) -> p h r", r=R)
                hi_rep = hi_t[:, bsl].unsqueeze(1).broadcast_to((P, H, R))
                lo_rep = lo_t[:, bsl].unsqueeze(1).broadcast_to((P, H, R))
                sv_rep = sv[:, bsl].unsqueeze(1).broadcast_to((P, H, R))
                nc.vector.tensor_tensor(
                    out=ohb[:, 0, :, :], in0=iota_rep, in1=hi_rep, op=OP.is_equal
                )
                nc.vector.tensor_tensor(
                    out=ohb[:, 1, :, :], in0=ohb[:, 0, :, :], in1=sv_rep,
                    op=OP.mult,
                )
                nc.vector.tensor_tensor(
                    out=olb[:], in0=iota_rep, in1=lo_rep, op=OP.is_equal
                )
                for j in range(R):
                    nc.tensor.matmul(
                        out=hist[:], lhsT=ohb[:, :, :, j], rhs=olb[:, :, j],
                        start=(tile_no == 0),
                        stop=(tile_no == n_tiles_total - 1),
                    )
                    tile_no += 1

        # ---- finalize: AllReduce the [128, 64] stats, then masked mean ----
        stats = fin_pool.tile([P, L], F32, tag="stats")
        nc.vector.tensor_copy(out=stats[:], in_=hist[:])
        nc.sync.dma_start(out=cc_in.ap(), in_=stats[:])
        nc.gpsimd.collective_compute(
            "AllReduce",
            OP.add,
            ins=[cc_in.ap().opt()],
            outs=[cc_out.ap().opt()],
            replica_groups=[list(range(CORES))],
        )
        cnt_t = fin_pool.tile([H, L], F32, tag="cnt_t")
        val_t = fin_pool.tile([H, L], F32, tag="val_t")
        cc_v = cc_out.ap()
        nc.sync.dma_start(out=cnt_t[:], in_=cc_v[0:H, :])
        nc.sync.dma_start(out=val_t[:], in_=cc_v[H : 2 * H, :])
        cnt = cnt_t[:]
        val = val_t[:]
        cntc = fin_pool.tile([H, L], F32, tag="cntc")
        gm = fin_pool.tile([H, L], F32, tag="gm")
        pres = fin_pool.tile([H, L], F32, tag="pres")
        nc.vector.tensor_scalar(
            out=cntc[:], in0=cnt, scalar1=1.0, scalar2=None, op0=OP.max
        )
        nc.vector.reciprocal(out=cntc[:], in_=cntc[:])
        nc.vector.tensor_tensor(out=gm[:], in0=val, in1=cntc[:], op=OP.mult)
        nc.vector.tensor_scalar(
            out=pres[:], in0=cnt, scalar1=0.0, scalar2=None, op0=OP.is_gt
        )
        nc.vector.tensor_tensor(out=gm[:], in0=gm[:], in1=pres[:], op=OP.mult)

        tot = fin_pool.tile([1, 1], F32, tag="tot")
        npres = fin_pool.tile([1, 1], F32, tag="npres")
        ans = fin_pool.tile([1, 1], F32, tag="ans")
        nc.gpsimd.tensor_reduce(out=tot[:], in_=gm[:], axis=AX.XYZWC, op=OP.add)
        nc.gpsimd.tensor_reduce(out=npres[:], in_=pres[:], axis=AX.XYZWC, op=OP.add)
        nc.vector.reciprocal(out=npres[:], in_=npres[:])
        nc.vector.tensor_tensor(out=ans[:], in0=tot[:], in1=npres[:], op=OP.mult)
        nc.sync.dma_start(out=out_d.ap(), in_=ans[:])

    nc.finalize()
    return nc


def make_in_maps(x, index, label, n_cores=CORES):
    n = x.shape[0]
    nc_sz = n // n_cores
    xs = np.ascontiguousarray(x, dtype=np.float32)
    iv = np.ascontiguousarray(index, dtype=np.int64).view(np.int32).reshape(n, 2)
    lv = np.ascontiguousarray(label, dtype=np.int64).view(np.int32).reshape(n, 2)
    maps = []
    for k in range(n_cores):
        sl = slice(k * nc_sz, (k + 1) * nc_sz)
        maps.append(
            {
                "x": np.ascontiguousarray(xs[sl]),
                "index": np.ascontiguousarray(iv[sl]),
                "label": np.ascontiguousarray(lv[sl]),
            }
        )
    return maps


_NC_CACHE = {}


def _get_nc(n_core, chunk_f):
    key = (n_core, chunk_f)
    if key not in _NC_CACHE:
        _NC_CACHE[key] = build_nc(n_core, chunk_f)
    return _NC_CACHE[key]


def kernel(x, index, label):
    n = x.shape[0]
    n_core = n // CORES
    nc = _get_nc(n_core, min(1024, n_core // P))
    in_maps = make_in_maps(x, index, label)
    res = run_bass_kernel_spmd(nc, in_maps, core_ids=list(range(CORES)))
    return np.float32(res.results[0]["out"][0, 0])


if __name__ == "__main__":
    rng = np.random.default_rng(0)
    n = 128 * 32 * CORES
    x = rng.standard_normal((n, 2), dtype=np.float32)
    index = rng.integers(0, G, n, dtype=np.int64)
    label = rng.integers(0, 2, n, dtype=np.int64)
    got = kernel(x, index, label)
    # numpy reference
    m = np.maximum(x[:, 0], x[:, 1])
    logz = m + np.log(np.exp(x[:, 0] - m) + np.exp(x[:, 1] - m))
    xt = x[np.arange(n), label]
    ce = logz - xt
    p = np.exp(xt - logz)
    s = (1.0 - p**2) * ce
    seg = np.zeros(G)
    cntr = np.zeros(G)
    np.add.at(seg, index, s)
    np.add.at(cntr, index, 1.0)
    pres = cntr > 0
    gmean = np.where(pres, seg / np.maximum(cntr, 1), 0.0)
    want = gmean.sum() / pres.sum()
    print("got", got, "want", want, "rel", abs(got - want) / abs(want))
